# revision 2
# baseline (speedup 1.0000x reference)
"""APPNP (10-hop propagation) on 8 TRN2 NeuronCores.

Strategy: shard destination nodes across the 8 cores (6250 each, padded to
6272 = 49*128). Per hop and per core: dma_gather the messages u[src] for the
core's in-edges (tokens pre-sorted by dst-block on the host), segment-sum them
on the TensorEngine via precomputed one-hot matrices (Q) streamed from HBM
into PSUM accumulators, apply the teleport update on the VectorEngine, and
AllGather the new u shards into a replicated HBM table for the next hop's
gather. Indices are int16, so the gather is split into a lo stream
(src_padded < 32768) and a hi stream (gathered from a +32768-row base view).
"""
import os
import sys

sys.path.insert(0, '/opt/trn_rl_repo')

import numpy as np

N = 50000
D = 64
E = 800000
K = 10
ALPHA = 0.1
C = 8                 # cores
NS = 6250             # real dst nodes per core
NSP = 6272            # padded (49 * 128)
NB = 49               # dst blocks per core
NP = C * NSP          # padded global rows = 50176
HALF = 32768
GB = 3                # dst blocks per gather/matmul group


def _host_prep(x, edge_index):
    src = np.asarray(edge_index[0], dtype=np.int64)
    dst = np.asarray(edge_index[1], dtype=np.int64)
    x = np.asarray(x, dtype=np.float32)

    deg = np.bincount(dst, minlength=N).astype(np.float64) + 1.0
    dinv = 1.0 / np.sqrt(deg)

    src_pad = (src // NS) * NSP + (src % NS)
    core = dst // NS
    dst_local = dst - core * NS
    block = dst_local // 128

    is_lo = src_pad < HALF
    # per (core, block, half) edge lists
    edges = {}
    for c in range(C):
        m = core == c
        for b in range(NB):
            mb = m & (block == b)
            for h, mh in ((0, mb & is_lo), (1, mb & ~is_lo)):
                edges[(c, b, h)] = (src_pad[mh], dst_local[mh] - b * 128)

    # equalize per (block, half) token counts across cores, pad to mult 128
    cnt = np.zeros((NB, 2), dtype=np.int64)
    for b in range(NB):
        for h in range(2):
            mx = max(len(edges[(c, b, h)][0]) for c in range(C))
            cnt[b, h] = max(128, ((mx + 127) // 128) * 128)

    TLO = int(cnt[:, 0].sum())
    THI = int(cnt[:, 1].sum())
    NCH_LO = TLO // 128
    NCH_HI = THI // 128
    TOTCH = NCH_LO + NCH_HI

    PAD_LO = np.int16(NS)                 # padded (always-zero) row, < HALF
    PAD_HI = np.int16(5 * NSP + NS - HALF)

    idx_lo = np.full((C, TLO), PAD_LO, dtype=np.int16)
    idx_hi = np.full((C, THI), PAD_HI, dtype=np.int16)
    Q = np.zeros((C, TOTCH, 128, 128), dtype=np.float32)

    # chunk index of the first chunk of each (block, half)
    ch0 = np.zeros((NB, 2), dtype=np.int64)
    off = 0
    for b in range(NB):
        ch0[b, 0] = off // 128
        off += cnt[b, 0]
    off = 0
    for b in range(NB):
        ch0[b, 1] = NCH_LO + off // 128
        off += cnt[b, 1]

    for c in range(C):
        off_lo = 0
        off_hi = 0
        for b in range(NB):
            for h in range(2):
                s, q = edges[(c, b, h)]
                n = len(s)
                if h == 0:
                    o = off_lo
                    idx_lo[c, o:o + n] = s.astype(np.int16)
                    off_lo += cnt[b, 0]
                else:
                    o = off_hi
                    idx_hi[c, o:o + n] = (s - HALF).astype(np.int16)
                    off_hi += cnt[b, 1]
                chb = ch0[b, h]
                sl = np.arange(n) + (o if h == 0 else o)
                # slot s -> chunk chb + s//128 (within this block-half), row s%128
                loc = np.arange(n) % 128
                chv = chb + (np.arange(n) + (o % 128) * 0) // 128
                # o is a multiple of 128, so slot position within block-half:
                pos = np.arange(n)
                Q[c, chb + pos // 128, pos % 128, q] = 1.0

    def wrap16(a):
        # token t -> partition t%16, column t//16, replicated to 128 partitions
        n = a.shape[-1]
        w = a.reshape(C, n // 16, 16).transpose(0, 2, 1)
        return np.tile(w, (1, 8, 1)).copy()

    idx_lo_w = wrap16(idx_lo)
    idx_hi_w = wrap16(idx_hi)

    # per-core node-level tensors, padded to NSP rows
    u0 = (dinv[:, None] * x).astype(np.float32)
    w_full = (0.9 * dinv * dinv).astype(np.float32)
    sq_full = np.sqrt(deg).astype(np.float32)

    def shard_pad(a2d):
        out = np.zeros((C, NSP, D), dtype=np.float32)
        for c in range(C):
            out[c, :NS] = a2d[c * NS:(c + 1) * NS]
        return out

    u0_s = shard_pad(u0)
    y0_s = shard_pad((ALPHA * dinv[:, None] * x).astype(np.float32))
    w_s = shard_pad(np.broadcast_to(w_full[:, None], (N, D)))
    sq_s = shard_pad(np.broadcast_to(sq_full[:, None], (N, D)))

    meta = dict(TLO=TLO, THI=THI, NCH_LO=NCH_LO, NCH_HI=NCH_HI,
                TOTCH=TOTCH, cnt=cnt, ch0=ch0)
    return meta, idx_lo_w, idx_hi_w, Q, u0_s, y0_s, w_s, sq_s


def _build_nc(meta):
    import concourse.bacc as bacc
    import concourse.mybir as mybir
    import concourse.tile as tile

    TLO, THI = meta['TLO'], meta['THI']
    NCH_LO, NCH_HI, TOTCH = meta['NCH_LO'], meta['NCH_HI'], meta['TOTCH']
    cnt, ch0 = meta['cnt'], meta['ch0']

    nc = bacc.Bacc(None, target_bir_lowering=False, num_devices=C)
    dt = mybir.dt.float32

    u0_d = nc.dram_tensor("u0", [NSP, D], dt, kind="ExternalInput")
    y0_d = nc.dram_tensor("y0", [NSP, D], dt, kind="ExternalInput")
    w_d = nc.dram_tensor("w", [NSP, D], dt, kind="ExternalInput")
    sq_d = nc.dram_tensor("sq", [NSP, D], dt, kind="ExternalInput")
    ilo_d = nc.dram_tensor("idx_lo", [128, TLO // 16], mybir.dt.int16, kind="ExternalInput")
    ihi_d = nc.dram_tensor("idx_hi", [128, THI // 16], mybir.dt.int16, kind="ExternalInput")
    q_d = nc.dram_tensor("Q", [TOTCH, 128, 128], dt, kind="ExternalInput")
    out_d = nc.dram_tensor("out", [NSP, D], dt, kind="ExternalOutput")

    bounce = nc.dram_tensor("bounce", [NSP, D], dt)
    ureps = [nc.dram_tensor(f"urep{i}", [NP, D], dt, addr_space="Shared")
             for i in range(2)]

    # block groups
    groups = []
    b = 0
    while b < NB:
        groups.append(list(range(b, min(b + GB, NB))))
        b += GB

    with tile.TileContext(nc) as tc:
        with (
            tc.tile_pool(name="res", bufs=1) as res,
            tc.tile_pool(name="mbuf", bufs=2) as mpool,
            tc.tile_pool(name="qbuf", bufs=2) as qpool,
            tc.tile_pool(name="psum", bufs=8, space="PSUM") as ppool,
        ):
            uA = res.tile([128, NB, D], dt, tag="uA")
            uB = res.tile([128, NB, D], dt, tag="uB")
            wt = res.tile([128, NB, D], dt, tag="wt")
            y0t = res.tile([128, NB, D], dt, tag="y0t")
            ilo_t = res.tile([128, TLO // 16], mybir.dt.int16, tag="ilo")
            ihi_t = res.tile([128, THI // 16], mybir.dt.int16, tag="ihi")

            def node_ap(dram):
                return dram[:].rearrange("(b p) f -> p b f", p=128)

            nc.sync.dma_start(uA[:], node_ap(u0_d))
            nc.sync.dma_start(wt[:], node_ap(w_d))
            nc.sync.dma_start(y0t[:], node_ap(y0_d))
            nc.sync.dma_start(ilo_t[:], ilo_d[:])
            nc.sync.dma_start(ihi_t[:], ihi_d[:])

            # initial AllGather of u0
            nc.sync.dma_start(bounce[:], u0_d[:])
            nc.gpsimd.collective_compute(
                "AllGather", mybir.AluOpType.bypass,
                replica_groups=[list(range(C))],
                ins=[bounce[:]], outs=[ureps[0][:]],
            )

            u_cur, u_nxt = uA, uB
            for h in range(K):
                urep = ureps[h % 2]
                lo_view = urep[0:HALF, :]
                hi_view = urep[HALF:NP, :]
                for g in groups:
                    b0, b1 = g[0], g[-1]
                    # token ranges for this group, both halves
                    slo0 = int(ch0[b0, 0]) * 128
                    nlo = int(cnt[g, 0].sum()) if hasattr(cnt, 'sum') else 0
                    nlo = int(sum(cnt[b, 0] for b in g))
                    shi0 = (int(ch0[b0, 1]) - NCH_LO) * 128
                    nhi = int(sum(cnt[b, 1] for b in g))

                    mlo = mpool.tile([128, nlo // 128, D], dt, tag="mlo")
                    mhi = mpool.tile([128, nhi // 128, D], dt, tag="mhi")
                    nc.gpsimd.dma_gather(
                        mlo[:], lo_view, ilo_t[:, slo0 // 16:(slo0 + nlo) // 16],
                        nlo, nlo, D, single_packet=False)
                    nc.gpsimd.dma_gather(
                        mhi[:], hi_view, ihi_t[:, shi0 // 16:(shi0 + nhi) // 16],
                        nhi, nhi, D, single_packet=False)

                    qlo = qpool.tile([128, (nlo // 128) * 128], dt, tag="qlo")
                    qhi = qpool.tile([128, (nhi // 128) * 128], dt, tag="qhi")
                    nc.sync.dma_start(
                        qlo[:], q_d[int(ch0[b0, 0]):int(ch0[b0, 0]) + nlo // 128]
                        .rearrange("c p q -> p c q"))
                    nc.sync.dma_start(
                        qhi[:], q_d[int(ch0[b0, 1]):int(ch0[b0, 1]) + nhi // 128]
                        .rearrange("c p q -> p c q"))

                    for b in g:
                        ps = ppool.tile([128, D], dt, tag="ps")
                        mms = []
                        nl = int(cnt[b, 0]) // 128
                        nh = int(cnt[b, 1]) // 128
                        cl0 = int(ch0[b, 0]) - int(ch0[b0, 0])
                        chh0 = int(ch0[b, 1]) - int(ch0[b0, 1])
                        sl_l = (int(ch0[b, 0]) * 128 - slo0) // 128
                        sl_h = ((int(ch0[b, 1]) - NCH_LO) * 128 - shi0) // 128
                        tot = nl + nh
                        k = 0
                        for j in range(nl):
                            nc.tensor.matmul(
                                ps[:], qlo[:, (cl0 + j) * 128:(cl0 + j + 1) * 128],
                                mlo[:, sl_l + j, :],
                                start=(k == 0), stop=(k == tot - 1))
                            k += 1
                        for j in range(nh):
                            nc.tensor.matmul(
                                ps[:], qhi[:, (chh0 + j) * 128:(chh0 + j + 1) * 128],
                                mhi[:, sl_h + j, :],
                                start=(k == 0), stop=(k == tot - 1))
                            k += 1
                        # u_new = w * (agg + u) + y0
                        nc.vector.tensor_tensor(
                            out=u_nxt[:, b, :], in0=ps[:], in1=u_cur[:, b, :],
                            op=mybir.AluOpType.add)
                        nc.vector.tensor_tensor(
                            out=u_nxt[:, b, :], in0=u_nxt[:, b, :], in1=wt[:, b, :],
                            op=mybir.AluOpType.mult)
                        nc.vector.tensor_tensor(
                            out=u_nxt[:, b, :], in0=u_nxt[:, b, :], in1=y0t[:, b, :],
                            op=mybir.AluOpType.add)

                if h < K - 1:
                    nc.sync.dma_start(node_ap(bounce), u_nxt[:])
                    nc.gpsimd.collective_compute(
                        "AllGather", mybir.AluOpType.bypass,
                        replica_groups=[list(range(C))],
                        ins=[bounce[:]], outs=[ureps[(h + 1) % 2][:]],
                    )
                u_cur, u_nxt = u_nxt, u_cur

            # epilogue: out = relu(u * sqrt(deg))
            sqt = res.tile([128, NB, D], dt, tag="sqt")
            nc.sync.dma_start(sqt[:], node_ap(sq_d))
            ot = res.tile([128, NB, D], dt, tag="ot")
            nc.vector.tensor_tensor(out=ot[:], in0=u_cur[:], in1=sqt[:],
                                    op=mybir.AluOpType.mult)
            nc.vector.tensor_scalar_max(out=ot[:], in0=ot[:], scalar1=0.0)
            nc.sync.dma_start(node_ap(out_d), ot[:])

    nc.compile()
    return nc


def kernel(x, edge_index):
    meta, idx_lo_w, idx_hi_w, Q, u0_s, y0_s, w_s, sq_s = _host_prep(x, edge_index)
    nc = _build_nc(meta)

    from concourse.bass_utils import run_bass_kernel_spmd

    in_maps = []
    for c in range(C):
        in_maps.append({
            "u0": u0_s[c], "y0": y0_s[c], "w": w_s[c], "sq": sq_s[c],
            "idx_lo": idx_lo_w[c], "idx_hi": idx_hi_w[c], "Q": Q[c],
        })

    ntff_dir = os.environ.get("APPNP_NTFF_DIR")
    if ntff_dir:
        from trn_agent_boot.trn_boot import _ntff_profile_via_ctypes
        hook = _ntff_profile_via_ctypes('/opt/axon/libaxon_pjrt.so')
        os.makedirs(ntff_dir, exist_ok=True)
        with hook(ntff_dir, None):
            res = run_bass_kernel_spmd(nc, in_maps, core_ids=list(range(C)))
    else:
        res = run_bass_kernel_spmd(nc, in_maps, core_ids=list(range(C)))

    out = np.empty((N, D), dtype=np.float32)
    for c in range(C):
        out[c * NS:(c + 1) * NS] = res.results[c]["out"][:NS]
    return out


# revision 4
# speedup vs baseline: 1.3278x; 1.3278x over previous
"""APPNP (10-hop propagation) on 8 TRN2 NeuronCores.

Strategy: shard destination nodes across the 8 cores (6250 each, padded to
6272 = 49*128). Per hop and per core: dma_gather the messages u[src] for the
core's in-edges (tokens pre-sorted by dst-block on the host), segment-sum them
on the TensorEngine via precomputed one-hot matrices (Q) streamed from HBM
into PSUM accumulators, apply the teleport update on the VectorEngine, and
AllGather the new u shards into a replicated HBM table for the next hop's
gather. Indices are int16, so the gather is split into a lo stream
(src_padded < 32768) and a hi stream (gathered from a +32768-row base view).
"""
import os
import sys

sys.path.insert(0, '/opt/trn_rl_repo')

import numpy as np

N = 50000
D = 64
E = 800000
K = 10
ALPHA = 0.1
C = 8                 # cores
NS = 6250             # real dst nodes per core
NSP = 6272            # padded (49 * 128)
NB = 49               # dst blocks per core
NP = C * NSP          # padded global rows = 50176
HALF = 32768
GB = 3                # dst blocks per gather/matmul group


def _host_prep(x, edge_index):
    src = np.asarray(edge_index[0], dtype=np.int64)
    dst = np.asarray(edge_index[1], dtype=np.int64)
    x = np.asarray(x, dtype=np.float32)

    deg = np.bincount(dst, minlength=N).astype(np.float64) + 1.0
    dinv = 1.0 / np.sqrt(deg)

    src_pad = (src // NS) * NSP + (src % NS)
    core = dst // NS
    dst_local = dst - core * NS
    block = dst_local // 128

    is_lo = src_pad < HALF
    # per (core, block, half) edge lists
    edges = {}
    for c in range(C):
        m = core == c
        for b in range(NB):
            mb = m & (block == b)
            for h, mh in ((0, mb & is_lo), (1, mb & ~is_lo)):
                edges[(c, b, h)] = (src_pad[mh], dst_local[mh] - b * 128)

    # equalize per (block, half) token counts across cores, pad to mult 128
    cnt = np.zeros((NB, 2), dtype=np.int64)
    for b in range(NB):
        for h in range(2):
            mx = max(len(edges[(c, b, h)][0]) for c in range(C))
            cnt[b, h] = max(128, ((mx + 127) // 128) * 128)

    TLO = int(cnt[:, 0].sum())
    THI = int(cnt[:, 1].sum())
    NCH_LO = TLO // 128
    NCH_HI = THI // 128
    TOTCH = NCH_LO + NCH_HI

    PAD_LO = np.int16(NS)                 # padded (always-zero) row, < HALF
    PAD_HI = np.int16(5 * NSP + NS - HALF)

    idx_lo = np.full((C, TLO), PAD_LO, dtype=np.int16)
    idx_hi = np.full((C, THI), PAD_HI, dtype=np.int16)
    Q = np.zeros((C, TOTCH, 128, 128), dtype=np.float32)

    # chunk index of the first chunk of each (block, half)
    ch0 = np.zeros((NB, 2), dtype=np.int64)
    off = 0
    for b in range(NB):
        ch0[b, 0] = off // 128
        off += cnt[b, 0]
    off = 0
    for b in range(NB):
        ch0[b, 1] = NCH_LO + off // 128
        off += cnt[b, 1]

    for c in range(C):
        off_lo = 0
        off_hi = 0
        for b in range(NB):
            for h in range(2):
                s, q = edges[(c, b, h)]
                n = len(s)
                if h == 0:
                    o = off_lo
                    idx_lo[c, o:o + n] = s.astype(np.int16)
                    off_lo += cnt[b, 0]
                else:
                    o = off_hi
                    idx_hi[c, o:o + n] = (s - HALF).astype(np.int16)
                    off_hi += cnt[b, 1]
                chb = ch0[b, h]
                sl = np.arange(n) + (o if h == 0 else o)
                # slot s -> chunk chb + s//128 (within this block-half), row s%128
                loc = np.arange(n) % 128
                chv = chb + (np.arange(n) + (o % 128) * 0) // 128
                # o is a multiple of 128, so slot position within block-half:
                pos = np.arange(n)
                Q[c, chb + pos // 128, pos % 128, q] = 1.0

    def wrap16(a):
        # token t -> partition t%16, column t//16, replicated to 128 partitions
        n = a.shape[-1]
        w = a.reshape(C, n // 16, 16).transpose(0, 2, 1)
        return np.tile(w, (1, 8, 1)).copy()

    idx_lo_w = wrap16(idx_lo)
    idx_hi_w = wrap16(idx_hi)

    # per-core node-level tensors, padded to NSP rows
    u0 = (dinv[:, None] * x).astype(np.float32)
    w_full = (0.9 * dinv * dinv).astype(np.float32)
    sq_full = np.sqrt(deg).astype(np.float32)

    def shard_pad(a2d):
        out = np.zeros((C, NSP, D), dtype=np.float32)
        for c in range(C):
            out[c, :NS] = a2d[c * NS:(c + 1) * NS]
        return out

    u0_s = shard_pad(u0)
    y0_s = shard_pad((ALPHA * dinv[:, None] * x).astype(np.float32))
    w_s = shard_pad(np.broadcast_to(w_full[:, None], (N, D)))
    sq_s = shard_pad(np.broadcast_to(sq_full[:, None], (N, D)))

    meta = dict(TLO=TLO, THI=THI, NCH_LO=NCH_LO, NCH_HI=NCH_HI,
                TOTCH=TOTCH, cnt=cnt, ch0=ch0)
    return meta, idx_lo_w, idx_hi_w, Q, u0_s, y0_s, w_s, sq_s


def _build_nc(meta):
    import concourse.bacc as bacc
    import concourse.mybir as mybir
    import concourse.tile as tile

    TLO, THI = meta['TLO'], meta['THI']
    NCH_LO, NCH_HI, TOTCH = meta['NCH_LO'], meta['NCH_HI'], meta['TOTCH']
    cnt, ch0 = meta['cnt'], meta['ch0']

    nc = bacc.Bacc(None, target_bir_lowering=False, num_devices=C, num_swdge_queues=4)
    dt = mybir.dt.float32

    u0_d = nc.dram_tensor("u0", [NSP, D], dt, kind="ExternalInput")
    y0_d = nc.dram_tensor("y0", [NSP, D], dt, kind="ExternalInput")
    w_d = nc.dram_tensor("w", [NSP, D], dt, kind="ExternalInput")
    sq_d = nc.dram_tensor("sq", [NSP, D], dt, kind="ExternalInput")
    ilo_d = nc.dram_tensor("idx_lo", [128, TLO // 16], mybir.dt.int16, kind="ExternalInput")
    ihi_d = nc.dram_tensor("idx_hi", [128, THI // 16], mybir.dt.int16, kind="ExternalInput")
    q_d = nc.dram_tensor("Q", [TOTCH, 128, 128], dt, kind="ExternalInput")
    out_d = nc.dram_tensor("out", [NSP, D], dt, kind="ExternalOutput")

    bounce = nc.dram_tensor("bounce", [NSP, D], dt)
    ureps = [nc.dram_tensor(f"urep{i}", [NP, D], dt, addr_space="Shared")
             for i in range(2)]

    # block groups
    groups = []
    b = 0
    while b < NB:
        groups.append(list(range(b, min(b + GB, NB))))
        b += GB

    with tile.TileContext(nc) as tc:
        with (
            tc.tile_pool(name="res", bufs=1) as res,
            tc.tile_pool(name="mbuf", bufs=2) as mpool,
            tc.tile_pool(name="qbuf", bufs=2) as qpool,
            tc.tile_pool(name="psum", bufs=8, space="PSUM") as ppool,
        ):
            uA = res.tile([128, NB, D], dt, tag="uA")
            uB = res.tile([128, NB, D], dt, tag="uB")
            wt = res.tile([128, NB, D], dt, tag="wt")
            y0t = res.tile([128, NB, D], dt, tag="y0t")
            ilo_t = res.tile([128, TLO // 16], mybir.dt.int16, tag="ilo")
            ihi_t = res.tile([128, THI // 16], mybir.dt.int16, tag="ihi")

            def node_ap(dram):
                return dram[:].rearrange("(b p) f -> p b f", p=128)

            nc.sync.dma_start(uA[:], node_ap(u0_d))
            nc.sync.dma_start(wt[:], node_ap(w_d))
            nc.sync.dma_start(y0t[:], node_ap(y0_d))
            nc.sync.dma_start(ilo_t[:], ilo_d[:])
            nc.sync.dma_start(ihi_t[:], ihi_d[:])

            # initial AllGather of u0
            nc.sync.dma_start(bounce[:], u0_d[:])
            nc.gpsimd.collective_compute(
                "AllGather", mybir.AluOpType.bypass,
                replica_groups=[list(range(C))],
                ins=[bounce[:]], outs=[ureps[0][:]],
            )

            u_cur, u_nxt = uA, uB
            qn = [0]
            for h in range(K):
                urep = ureps[h % 2]
                lo_view = urep[0:HALF, :]
                hi_view = urep[HALF:NP, :]
                for g in groups:
                    b0, b1 = g[0], g[-1]
                    # token ranges for this group, both halves
                    slo0 = int(ch0[b0, 0]) * 128
                    nlo = int(cnt[g, 0].sum()) if hasattr(cnt, 'sum') else 0
                    nlo = int(sum(cnt[b, 0] for b in g))
                    shi0 = (int(ch0[b0, 1]) - NCH_LO) * 128
                    nhi = int(sum(cnt[b, 1] for b in g))

                    mlo = mpool.tile([128, nlo // 128, D], dt, tag="mlo")
                    mhi = mpool.tile([128, nhi // 128, D], dt, tag="mhi")
                    nc.gpsimd.dma_gather(
                        mlo[:], lo_view, ilo_t[:, slo0 // 16:(slo0 + nlo) // 16],
                        nlo, nlo, D, single_packet=False, queue_num=qn[0] % 4)
                    qn[0] += 1
                    nc.gpsimd.dma_gather(
                        mhi[:], hi_view, ihi_t[:, shi0 // 16:(shi0 + nhi) // 16],
                        nhi, nhi, D, single_packet=False, queue_num=qn[0] % 4)
                    qn[0] += 1

                    qlo = qpool.tile([128, (nlo // 128) * 128], dt, tag="qlo")
                    qhi = qpool.tile([128, (nhi // 128) * 128], dt, tag="qhi")
                    nc.sync.dma_start(
                        qlo[:], q_d[int(ch0[b0, 0]):int(ch0[b0, 0]) + nlo // 128]
                        .rearrange("c p q -> p c q"))
                    nc.sync.dma_start(
                        qhi[:], q_d[int(ch0[b0, 1]):int(ch0[b0, 1]) + nhi // 128]
                        .rearrange("c p q -> p c q"))

                    for b in g:
                        ps = ppool.tile([128, D], dt, tag="ps")
                        mms = []
                        nl = int(cnt[b, 0]) // 128
                        nh = int(cnt[b, 1]) // 128
                        cl0 = int(ch0[b, 0]) - int(ch0[b0, 0])
                        chh0 = int(ch0[b, 1]) - int(ch0[b0, 1])
                        sl_l = (int(ch0[b, 0]) * 128 - slo0) // 128
                        sl_h = ((int(ch0[b, 1]) - NCH_LO) * 128 - shi0) // 128
                        tot = nl + nh
                        k = 0
                        for j in range(nl):
                            nc.tensor.matmul(
                                ps[:], qlo[:, (cl0 + j) * 128:(cl0 + j + 1) * 128],
                                mlo[:, sl_l + j, :],
                                start=(k == 0), stop=(k == tot - 1))
                            k += 1
                        for j in range(nh):
                            nc.tensor.matmul(
                                ps[:], qhi[:, (chh0 + j) * 128:(chh0 + j + 1) * 128],
                                mhi[:, sl_h + j, :],
                                start=(k == 0), stop=(k == tot - 1))
                            k += 1
                        # u_new = w * (agg + u) + y0
                        nc.vector.tensor_tensor(
                            out=u_nxt[:, b, :], in0=ps[:], in1=u_cur[:, b, :],
                            op=mybir.AluOpType.add)
                        nc.vector.tensor_tensor(
                            out=u_nxt[:, b, :], in0=u_nxt[:, b, :], in1=wt[:, b, :],
                            op=mybir.AluOpType.mult)
                        nc.vector.tensor_tensor(
                            out=u_nxt[:, b, :], in0=u_nxt[:, b, :], in1=y0t[:, b, :],
                            op=mybir.AluOpType.add)

                if h < K - 1:
                    nc.sync.dma_start(node_ap(bounce), u_nxt[:])
                    nc.gpsimd.collective_compute(
                        "AllGather", mybir.AluOpType.bypass,
                        replica_groups=[list(range(C))],
                        ins=[bounce[:]], outs=[ureps[(h + 1) % 2][:]],
                    )
                u_cur, u_nxt = u_nxt, u_cur

            # epilogue: out = relu(u * sqrt(deg))
            sqt = res.tile([128, NB, D], dt, tag="sqt")
            nc.sync.dma_start(sqt[:], node_ap(sq_d))
            ot = res.tile([128, NB, D], dt, tag="ot")
            nc.vector.tensor_tensor(out=ot[:], in0=u_cur[:], in1=sqt[:],
                                    op=mybir.AluOpType.mult)
            nc.vector.tensor_scalar_max(out=ot[:], in0=ot[:], scalar1=0.0)
            nc.sync.dma_start(node_ap(out_d), ot[:])

    nc.compile()
    return nc


def kernel(x, edge_index):
    meta, idx_lo_w, idx_hi_w, Q, u0_s, y0_s, w_s, sq_s = _host_prep(x, edge_index)
    nc = _build_nc(meta)

    from concourse.bass_utils import run_bass_kernel_spmd

    in_maps = []
    for c in range(C):
        in_maps.append({
            "u0": u0_s[c], "y0": y0_s[c], "w": w_s[c], "sq": sq_s[c],
            "idx_lo": idx_lo_w[c], "idx_hi": idx_hi_w[c], "Q": Q[c],
        })

    ntff_dir = os.environ.get("APPNP_NTFF_DIR")
    if ntff_dir:
        from trn_agent_boot.trn_boot import _ntff_profile_via_ctypes
        hook = _ntff_profile_via_ctypes('/opt/axon/libaxon_pjrt.so')
        os.makedirs(ntff_dir, exist_ok=True)
        with hook(ntff_dir, None):
            res = run_bass_kernel_spmd(nc, in_maps, core_ids=list(range(C)))
    else:
        res = run_bass_kernel_spmd(nc, in_maps, core_ids=list(range(C)))

    out = np.empty((N, D), dtype=np.float32)
    for c in range(C):
        out[c * NS:(c + 1) * NS] = res.results[c]["out"][:NS]
    return out


# revision 5
# speedup vs baseline: 1.4717x; 1.1084x over previous
"""APPNP (10-hop propagation) on 8 TRN2 NeuronCores.

Strategy: shard destination nodes across the 8 cores (6250 each, padded to
6272 = 49*128). Per hop and per core: dma_gather the messages u[src] for the
core's in-edges (tokens pre-sorted by dst-block on the host), segment-sum them
on the TensorEngine via precomputed one-hot matrices (Q) streamed from HBM
into PSUM accumulators, apply the teleport update on the VectorEngine, and
AllGather the new u shards into a replicated HBM table for the next hop's
gather. Indices are int16, so the gather is split into a lo stream
(src_padded < 32768) and a hi stream (gathered from a +32768-row base view).
"""
import os
import sys

sys.path.insert(0, '/opt/trn_rl_repo')

import numpy as np

N = 50000
D = 64
E = 800000
K = 10
ALPHA = 0.1
C = 8                 # cores
NS = 6250             # real dst nodes per core
NSP = 6272            # padded (49 * 128)
NB = 49               # dst blocks per core
NP = C * NSP          # padded global rows = 50176
HALF = 32768
GB = 3                # dst blocks per gather/matmul group


def _host_prep(x, edge_index):
    src = np.asarray(edge_index[0], dtype=np.int64)
    dst = np.asarray(edge_index[1], dtype=np.int64)
    x = np.asarray(x, dtype=np.float32)

    deg = np.bincount(dst, minlength=N).astype(np.float64) + 1.0
    dinv = 1.0 / np.sqrt(deg)

    src_pad = (src // NS) * NSP + (src % NS)
    core = dst // NS
    dst_local = dst - core * NS
    block = dst_local // 128

    is_lo = src_pad < HALF
    # per (core, block, half) edge lists
    edges = {}
    for c in range(C):
        m = core == c
        for b in range(NB):
            mb = m & (block == b)
            for h, mh in ((0, mb & is_lo), (1, mb & ~is_lo)):
                edges[(c, b, h)] = (src_pad[mh], dst_local[mh] - b * 128)

    # equalize per (block, half) token counts across cores, pad to mult 128
    cnt = np.zeros((NB, 2), dtype=np.int64)
    for b in range(NB):
        for h in range(2):
            mx = max(len(edges[(c, b, h)][0]) for c in range(C))
            cnt[b, h] = max(128, ((mx + 127) // 128) * 128)

    TLO = int(cnt[:, 0].sum())
    THI = int(cnt[:, 1].sum())
    NCH_LO = TLO // 128
    NCH_HI = THI // 128
    TOTCH = NCH_LO + NCH_HI

    PAD_LO = np.int16(NS)                 # padded (always-zero) row, < HALF
    PAD_HI = np.int16(5 * NSP + NS - HALF)

    idx_lo = np.full((C, TLO), PAD_LO, dtype=np.int16)
    idx_hi = np.full((C, THI), PAD_HI, dtype=np.int16)
    Q = np.zeros((C, TOTCH, 128, 128), dtype=np.float32)

    # chunk index of the first chunk of each (block, half)
    ch0 = np.zeros((NB, 2), dtype=np.int64)
    off = 0
    for b in range(NB):
        ch0[b, 0] = off // 128
        off += cnt[b, 0]
    off = 0
    for b in range(NB):
        ch0[b, 1] = NCH_LO + off // 128
        off += cnt[b, 1]

    for c in range(C):
        off_lo = 0
        off_hi = 0
        for b in range(NB):
            for h in range(2):
                s, q = edges[(c, b, h)]
                n = len(s)
                if h == 0:
                    o = off_lo
                    idx_lo[c, o:o + n] = s.astype(np.int16)
                    off_lo += cnt[b, 0]
                else:
                    o = off_hi
                    idx_hi[c, o:o + n] = (s - HALF).astype(np.int16)
                    off_hi += cnt[b, 1]
                chb = ch0[b, h]
                sl = np.arange(n) + (o if h == 0 else o)
                # slot s -> chunk chb + s//128 (within this block-half), row s%128
                loc = np.arange(n) % 128
                chv = chb + (np.arange(n) + (o % 128) * 0) // 128
                # o is a multiple of 128, so slot position within block-half:
                pos = np.arange(n)
                Q[c, chb + pos // 128, pos % 128, q] = 1.0

    def wrap16(a):
        # token t -> partition t%16, column t//16, replicated to 128 partitions
        n = a.shape[-1]
        w = a.reshape(C, n // 16, 16).transpose(0, 2, 1)
        return np.tile(w, (1, 8, 1)).copy()

    idx_lo_w = wrap16(idx_lo)
    idx_hi_w = wrap16(idx_hi)

    # per-core node-level tensors, padded to NSP rows
    u0 = (dinv[:, None] * x).astype(np.float32)
    w_full = (0.9 * dinv * dinv).astype(np.float32)
    sq_full = np.sqrt(deg).astype(np.float32)

    def shard_pad(a2d):
        out = np.zeros((C, NSP, D), dtype=np.float32)
        for c in range(C):
            out[c, :NS] = a2d[c * NS:(c + 1) * NS]
        return out

    u0_s = shard_pad(u0)
    y0_s = shard_pad((ALPHA * dinv[:, None] * x).astype(np.float32))
    w_s = shard_pad(np.broadcast_to(w_full[:, None], (N, D)))
    sq_s = shard_pad(np.broadcast_to(sq_full[:, None], (N, D)))

    meta = dict(TLO=TLO, THI=THI, NCH_LO=NCH_LO, NCH_HI=NCH_HI,
                TOTCH=TOTCH, cnt=cnt, ch0=ch0)
    return meta, idx_lo_w, idx_hi_w, Q, u0_s, y0_s, w_s, sq_s


def _build_nc(meta):
    import concourse.bacc as bacc
    import concourse.mybir as mybir
    import concourse.tile as tile

    TLO, THI = meta['TLO'], meta['THI']
    NCH_LO, NCH_HI, TOTCH = meta['NCH_LO'], meta['NCH_HI'], meta['TOTCH']
    cnt, ch0 = meta['cnt'], meta['ch0']

    nc = bacc.Bacc(None, target_bir_lowering=False, num_devices=C, num_swdge_queues=4)
    dt = mybir.dt.float32

    u0_d = nc.dram_tensor("u0", [NSP, D], dt, kind="ExternalInput")
    y0_d = nc.dram_tensor("y0", [NSP, D], dt, kind="ExternalInput")
    w_d = nc.dram_tensor("w", [NSP, D], dt, kind="ExternalInput")
    sq_d = nc.dram_tensor("sq", [NSP, D], dt, kind="ExternalInput")
    ilo_d = nc.dram_tensor("idx_lo", [128, TLO // 16], mybir.dt.int16, kind="ExternalInput")
    ihi_d = nc.dram_tensor("idx_hi", [128, THI // 16], mybir.dt.int16, kind="ExternalInput")
    q_d = nc.dram_tensor("Q", [TOTCH, 128, 128], dt, kind="ExternalInput")
    out_d = nc.dram_tensor("out", [NSP, D], dt, kind="ExternalOutput")

    bounce = nc.dram_tensor("bounce", [NSP, D], dt)
    ureps = [nc.dram_tensor(f"urep{i}", [NP, D], dt, addr_space="Shared")
             for i in range(2)]

    # block groups
    groups = []
    b = 0
    while b < NB:
        groups.append(list(range(b, min(b + GB, NB))))
        b += GB

    with tile.TileContext(nc) as tc:
        with (
            tc.tile_pool(name="res", bufs=1) as res,
            tc.tile_pool(name="mbuf", bufs=4) as mpool,
            tc.tile_pool(name="qbuf", bufs=2) as qpool,
            tc.tile_pool(name="psum", bufs=8, space="PSUM") as ppool,
        ):
            uA = res.tile([128, NB, D], dt, tag="uA")
            uB = res.tile([128, NB, D], dt, tag="uB")
            wt = res.tile([128, NB, D], dt, tag="wt")
            y0t = res.tile([128, NB, D], dt, tag="y0t")
            ilo_t = res.tile([128, TLO // 16], mybir.dt.int16, tag="ilo")
            ihi_t = res.tile([128, THI // 16], mybir.dt.int16, tag="ihi")

            def node_ap(dram):
                return dram[:].rearrange("(b p) f -> p b f", p=128)

            nc.sync.dma_start(uA[:], node_ap(u0_d))
            nc.sync.dma_start(wt[:], node_ap(w_d))
            nc.sync.dma_start(y0t[:], node_ap(y0_d))
            nc.sync.dma_start(ilo_t[:], ilo_d[:])
            nc.sync.dma_start(ihi_t[:], ihi_d[:])

            # initial AllGather of u0
            nc.sync.dma_start(bounce[:], u0_d[:])
            nc.gpsimd.collective_compute(
                "AllGather", mybir.AluOpType.bypass,
                replica_groups=[list(range(C))],
                ins=[bounce[:]], outs=[ureps[0][:]],
            )

            u_cur, u_nxt = uA, uB
            qn = [0]
            for h in range(K):
                urep = ureps[h % 2]
                lo_view = urep[0:HALF, :]
                hi_view = urep[HALF:NP, :]
                for g in groups:
                    b0, b1 = g[0], g[-1]
                    # token ranges for this group, both halves
                    slo0 = int(ch0[b0, 0]) * 128
                    nlo = int(cnt[g, 0].sum()) if hasattr(cnt, 'sum') else 0
                    nlo = int(sum(cnt[b, 0] for b in g))
                    shi0 = (int(ch0[b0, 1]) - NCH_LO) * 128
                    nhi = int(sum(cnt[b, 1] for b in g))

                    mlo = mpool.tile([128, nlo // 128, D], dt, tag="mlo")
                    mhi = mpool.tile([128, nhi // 128, D], dt, tag="mhi")
                    nc.gpsimd.dma_gather(
                        mlo[:], lo_view, ilo_t[:, slo0 // 16:(slo0 + nlo) // 16],
                        nlo, nlo, D, single_packet=False, queue_num=qn[0] % 4)
                    qn[0] += 1
                    nc.gpsimd.dma_gather(
                        mhi[:], hi_view, ihi_t[:, shi0 // 16:(shi0 + nhi) // 16],
                        nhi, nhi, D, single_packet=False, queue_num=qn[0] % 4)
                    qn[0] += 1

                    qlo = qpool.tile([128, (nlo // 128) * 128], dt, tag="qlo")
                    qhi = qpool.tile([128, (nhi // 128) * 128], dt, tag="qhi")
                    nc.sync.dma_start(
                        qlo[:], q_d[int(ch0[b0, 0]):int(ch0[b0, 0]) + nlo // 128]
                        .rearrange("c p q -> p c q"))
                    nc.sync.dma_start(
                        qhi[:], q_d[int(ch0[b0, 1]):int(ch0[b0, 1]) + nhi // 128]
                        .rearrange("c p q -> p c q"))

                    for b in g:
                        ps = ppool.tile([128, D], dt, tag="ps")
                        mms = []
                        nl = int(cnt[b, 0]) // 128
                        nh = int(cnt[b, 1]) // 128
                        cl0 = int(ch0[b, 0]) - int(ch0[b0, 0])
                        chh0 = int(ch0[b, 1]) - int(ch0[b0, 1])
                        sl_l = (int(ch0[b, 0]) * 128 - slo0) // 128
                        sl_h = ((int(ch0[b, 1]) - NCH_LO) * 128 - shi0) // 128
                        tot = nl + nh
                        k = 0
                        for j in range(nl):
                            nc.tensor.matmul(
                                ps[:], qlo[:, (cl0 + j) * 128:(cl0 + j + 1) * 128],
                                mlo[:, sl_l + j, :],
                                start=(k == 0), stop=(k == tot - 1))
                            k += 1
                        for j in range(nh):
                            nc.tensor.matmul(
                                ps[:], qhi[:, (chh0 + j) * 128:(chh0 + j + 1) * 128],
                                mhi[:, sl_h + j, :],
                                start=(k == 0), stop=(k == tot - 1))
                            k += 1
                        # u_new = w * (agg + u) + y0
                        nc.vector.tensor_tensor(
                            out=u_nxt[:, b, :], in0=ps[:], in1=u_cur[:, b, :],
                            op=mybir.AluOpType.add)
                        nc.vector.tensor_tensor(
                            out=u_nxt[:, b, :], in0=u_nxt[:, b, :], in1=wt[:, b, :],
                            op=mybir.AluOpType.mult)
                        nc.vector.tensor_tensor(
                            out=u_nxt[:, b, :], in0=u_nxt[:, b, :], in1=y0t[:, b, :],
                            op=mybir.AluOpType.add)

                if h < K - 1:
                    nc.sync.dma_start(node_ap(bounce), u_nxt[:])
                    nc.gpsimd.collective_compute(
                        "AllGather", mybir.AluOpType.bypass,
                        replica_groups=[list(range(C))],
                        ins=[bounce[:]], outs=[ureps[(h + 1) % 2][:]],
                    )
                u_cur, u_nxt = u_nxt, u_cur

            # epilogue: out = relu(u * sqrt(deg))
            sqt = res.tile([128, NB, D], dt, tag="sqt")
            nc.sync.dma_start(sqt[:], node_ap(sq_d))
            ot = res.tile([128, NB, D], dt, tag="ot")
            nc.vector.tensor_tensor(out=ot[:], in0=u_cur[:], in1=sqt[:],
                                    op=mybir.AluOpType.mult)
            nc.vector.tensor_scalar_max(out=ot[:], in0=ot[:], scalar1=0.0)
            nc.sync.dma_start(node_ap(out_d), ot[:])

    nc.compile()
    return nc


def kernel(x, edge_index):
    meta, idx_lo_w, idx_hi_w, Q, u0_s, y0_s, w_s, sq_s = _host_prep(x, edge_index)
    nc = _build_nc(meta)

    from concourse.bass_utils import run_bass_kernel_spmd

    in_maps = []
    for c in range(C):
        in_maps.append({
            "u0": u0_s[c], "y0": y0_s[c], "w": w_s[c], "sq": sq_s[c],
            "idx_lo": idx_lo_w[c], "idx_hi": idx_hi_w[c], "Q": Q[c],
        })

    ntff_dir = os.environ.get("APPNP_NTFF_DIR")
    if ntff_dir:
        from trn_agent_boot.trn_boot import _ntff_profile_via_ctypes
        hook = _ntff_profile_via_ctypes('/opt/axon/libaxon_pjrt.so')
        os.makedirs(ntff_dir, exist_ok=True)
        with hook(ntff_dir, None):
            res = run_bass_kernel_spmd(nc, in_maps, core_ids=list(range(C)))
    else:
        res = run_bass_kernel_spmd(nc, in_maps, core_ids=list(range(C)))

    out = np.empty((N, D), dtype=np.float32)
    for c in range(C):
        out[c * NS:(c + 1) * NS] = res.results[c]["out"][:NS]
    return out
